# revision 8
# baseline (speedup 1.0000x reference)
"""AttentionBlock (GroupNorm + single-head self-attention + residual) on 8 trn2 cores.

Data-parallel over batch: B=16 -> 2 batch elements per core. Per batch element
(C=512 channels, T=H*W=1024 tokens), everything is kept in channel-major
[C, T] layouts so the whole chain needs zero activation transposes:

  h  = groupnorm(x)                 [C, T]   (bn_stats per channel + block-diag
                                              matmul for cross-partition group agg)
  W  = wq^T @ wk                    [C, C]   (once per core; uses native [O,C] layout)
  u  = W^T @ h  (+ gk := wk^T bq)   [C, T]
  sT = h^T(j) @ u                   [T, T]   scores transposed: [key j, query i]
  eT = exp(sT * C^-1/2)             [T, T]   unnormalized softmax numerator
  Z  = ones^T @ eT                  per-query sums, broadcast to 128 partitions
  oT = (v^T @ eT) * (1/Z) + bv      [C, T]   v = h^T @ wv^T
  fT = wo^T' @ oT                   [C, T]
  y  = x + fT + bo
"""

import numpy as np

B, C, HW = 16, 512, 1024
H = W_SP = 32
G = 16  # channels per group (num_groups=32)
NCORES = 8
BL = B // NCORES  # 2 batch elements per core
CT = C // 128  # 4 channel tiles
TT = HW // 128  # 8 token tiles
CH = HW // 512  # 2 free-dim chunks of 512
EPS = 1e-5
SC = float(C) ** -0.5


def build_program(nc, reps=1):
    import concourse.bass as bass
    import concourse.tile as tile
    from concourse import mybir

    f32 = mybir.dt.float32
    AF = mybir.ActivationFunctionType
    OP = mybir.AluOpType

    x_d = nc.dram_tensor("x", [BL, C, HW], f32, kind="ExternalInput")
    wq_d = nc.dram_tensor("wq", [C, C], f32, kind="ExternalInput")
    wk_d = nc.dram_tensor("wk", [C, C], f32, kind="ExternalInput")
    wv_d = nc.dram_tensor("wv", [C, C], f32, kind="ExternalInput")
    wo_d = nc.dram_tensor("wo", [C, C], f32, kind="ExternalInput")
    nw_d = nc.dram_tensor("norm_w", [C, 1], f32, kind="ExternalInput")
    nb_d = nc.dram_tensor("norm_b", [C, 1], f32, kind="ExternalInput")
    bq_d = nc.dram_tensor("bq", [C, 1], f32, kind="ExternalInput")
    bk_d = nc.dram_tensor("bk", [C, 1], f32, kind="ExternalInput")
    bv_d = nc.dram_tensor("bv", [C, 1], f32, kind="ExternalInput")
    bo_d = nc.dram_tensor("bo", [C, 1], f32, kind="ExternalInput")
    bd_d = nc.dram_tensor("bd16", [128, 128], f32, kind="ExternalInput")
    id_d = nc.dram_tensor("ident", [128, 128], f32, kind="ExternalInput")
    y_d = nc.dram_tensor("y", [BL, C, HW], f32, kind="ExternalOutput")

    with tile.TileContext(nc) as tc:
        with (
            tc.tile_pool(name="persist", bufs=1) as persist,
            tc.tile_pool(name="wtmp", bufs=1) as wtmp,
            tc.tile_pool(name="xin", bufs=2) as xin,
            tc.tile_pool(name="big", bufs=1) as big,
            tc.tile_pool(name="yout", bufs=3) as yout,
            tc.tile_pool(name="small", bufs=2) as small,
            tc.tile_pool(name="ps_score", bufs=2, space="PSUM") as ps_score,
            tc.tile_pool(name="ps_acc", bufs=3, space="PSUM") as ps_acc,
        ):
            # ---------------- startup: weights + constants ----------------
            wq_sb = wtmp.tile([128, CT, C], f32)
            wk_sb = wtmp.tile([128, CT, C], f32)
            wv_sb = wtmp.tile([128, CT, C], f32)
            wo_sb = wtmp.tile([128, CT, C], f32)
            for oi in range(CT):
                nc.sync.dma_start(out=wq_sb[:, oi, :], in_=wq_d[oi * 128:(oi + 1) * 128, :])
                nc.sync.dma_start(out=wk_sb[:, oi, :], in_=wk_d[oi * 128:(oi + 1) * 128, :])
                nc.sync.dma_start(out=wv_sb[:, oi, :], in_=wv_d[oi * 128:(oi + 1) * 128, :])
                nc.sync.dma_start(out=wo_sb[:, oi, :], in_=wo_d[oi * 128:(oi + 1) * 128, :])

            vecs = persist.tile([128, CT, 6], f32)  # norm_w, norm_b, bq, bv, bo, (spare)
            for ci in range(CT):
                sl = slice(ci * 128, (ci + 1) * 128)
                nc.sync.dma_start(out=vecs[:, ci, 0:1], in_=nw_d[sl, :])
                nc.sync.dma_start(out=vecs[:, ci, 1:2], in_=nb_d[sl, :])
                nc.sync.dma_start(out=vecs[:, ci, 2:3], in_=bq_d[sl, :])
                nc.sync.dma_start(out=vecs[:, ci, 3:4], in_=bv_d[sl, :])
                nc.sync.dma_start(out=vecs[:, ci, 4:5], in_=bo_d[sl, :])
                nc.sync.dma_start(out=vecs[:, ci, 5:6], in_=bk_d[sl, :])
            bd_sb = persist.tile([128, 128], f32)
            nc.sync.dma_start(out=bd_sb, in_=bd_d[:, :])
            id_sb = persist.tile([128, 128], f32)
            nc.sync.dma_start(out=id_sb, in_=id_d[:, :])
            ones_sb = persist.tile([128, 128], f32)
            nc.vector.memset(ones_sb, 1.0)
            eps_sb = persist.tile([128, 1], f32)
            nc.vector.memset(eps_sb, EPS)

            # W = wq^T @ wk  [ci, cj]  (contraction over output channel o)
            W_t = persist.tile([128, CT, C], f32)
            for ci in range(CT):
                ps_w = ps_acc.tile([128, 512], f32, tag="acc", name="ps_w")
                for oi in range(CT):
                    nc.tensor.matmul(
                        ps_w, wq_sb[:, oi, ci * 128:(ci + 1) * 128], wk_sb[:, oi, :],
                        start=(oi == 0), stop=(oi == CT - 1),
                    )
                nc.scalar.copy(out=W_t[:, ci, :], in_=ps_w)

            # wvT / woT via PE transpose of 128x128 blocks
            wvT_t = persist.tile([128, CT, C], f32)
            woT_t = persist.tile([128, CT, C], f32)
            for src, dst in ((wv_sb, wvT_t), (wo_sb, woT_t)):
                for oi in range(CT):
                    for ci in range(CT):
                        ps_t = ps_acc.tile([128, 128], f32, tag="acc", name="ps_t")
                        nc.tensor.transpose(
                            ps_t, src[:, oi, ci * 128:(ci + 1) * 128], id_sb
                        )
                        nc.scalar.copy(
                            out=dst[:, ci, oi * 128:(oi + 1) * 128], in_=ps_t
                        )

            # gk = wk^T @ bq  (per-key score bias correction), [C] as [128, CT]
            gk_sb = persist.tile([128, CT], f32)
            for ci in range(CT):
                ps_g = ps_acc.tile([128, 1], f32, tag="acc", name="ps_g")
                for oi in range(CT):
                    nc.tensor.matmul(
                        ps_g, wk_sb[:, oi, ci * 128:(ci + 1) * 128], vecs[:, oi, 2:3],
                        start=(oi == 0), stop=(oi == CT - 1),
                    )
                nc.scalar.copy(out=gk_sb[:, ci:ci + 1], in_=ps_g)

            # ---------------- per batch element ----------------
            for b in [b for _ in range(reps) for b in range(BL)]:
                x_t = xin.tile([128, CT, HW], f32, name="x_t")
                for ci in range(CT):
                    nc.sync.dma_start(
                        out=x_t[:, ci, :], in_=x_d[b, ci * 128:(ci + 1) * 128, :]
                    )

                # --- group norm ---
                h_t = big.tile([128, CT, HW], f32, name="h_t")
                for ci in range(CT):
                    stats = small.tile([128, 2, 6], f32, name="stats")
                    for s in range(2):
                        nc.vector.bn_stats(
                            out=stats[:, s, :], in_=x_t[:, ci, s * 512:(s + 1) * 512]
                        )
                    mv = small.tile([128, 2], f32, name="mv")
                    nc.vector.bn_aggr(out=mv, in_=stats)
                    st2 = small.tile([128, 2], f32, name="st2")
                    nc.vector.tensor_copy(out=st2[:, 0:1], in_=mv[:, 0:1])
                    nc.vector.tensor_mul(out=st2[:, 1:2], in0=mv[:, 0:1], in1=mv[:, 0:1])
                    nc.vector.tensor_add(out=st2[:, 1:2], in0=st2[:, 1:2], in1=mv[:, 1:2])
                    ps_st = ps_acc.tile([128, 2], f32, tag="acc", name="ps_st")
                    nc.tensor.matmul(ps_st, bd_sb, st2, start=True, stop=True)
                    mug = small.tile([128, 1], f32, name="mug")
                    nc.vector.tensor_copy(out=mug, in_=ps_st[:, 0:1])
                    tv = small.tile([128, 1], f32, name="tv")
                    nc.vector.tensor_mul(out=tv, in0=mug, in1=mug)
                    nc.vector.tensor_sub(out=tv, in0=ps_st[:, 1:2], in1=tv)
                    nc.scalar.activation(out=tv, in_=tv, func=AF.Sqrt, bias=eps_sb, scale=1.0)
                    nc.vector.reciprocal(out=tv, in_=tv)
                    sc_c = small.tile([128, 1], f32, name="sc_c")
                    nc.vector.tensor_mul(out=sc_c, in0=tv, in1=vecs[:, ci, 0:1])
                    bi_c = small.tile([128, 1], f32, name="bi_c")
                    nc.vector.tensor_mul(out=bi_c, in0=mug, in1=sc_c)
                    nc.vector.tensor_sub(out=bi_c, in0=vecs[:, ci, 1:2], in1=bi_c)
                    nc.vector.tensor_scalar(
                        out=h_t[:, ci, :], in0=x_t[:, ci, :],
                        scalar1=sc_c, scalar2=bi_c, op0=OP.mult, op1=OP.add,
                    )
                    # x_t becomes (x + bo) for the final residual
                    nc.scalar.activation(
                        out=x_t[:, ci, :], in_=x_t[:, ci, :], func=AF.Identity,
                        bias=vecs[:, ci, 4:5], scale=1.0,
                    )

                # --- v = h^T @ wv^T  [token, c_out] ---
                v_t = big.tile([128, TT, 512], f32, name="v_t")
                for tt in range(TT):
                    ps_v = ps_acc.tile([128, 512], f32, tag="acc", name="ps_v")
                    for ci in range(CT):
                        nc.tensor.matmul(
                            ps_v, h_t[:, ci, tt * 128:(tt + 1) * 128], wvT_t[:, ci, :],
                            start=(ci == 0), stop=(ci == CT - 1),
                        )
                    nc.scalar.copy(out=v_t[:, tt, :], in_=ps_v)

                # --- u = W^T @ h (+gk)  [cj, query i] ---
                u_t = big.tile([128, CT, HW], f32, name="u_t")
                for cj in range(CT):
                    for ch in range(CH):
                        ps_u = ps_acc.tile([128, 512], f32, tag="acc", name="ps_u")
                        for ci in range(CT):
                            nc.tensor.matmul(
                                ps_u, W_t[:, ci, cj * 128:(cj + 1) * 128],
                                h_t[:, ci, ch * 512:(ch + 1) * 512],
                                start=(ci == 0), stop=(ci == CT - 1),
                            )
                        nc.vector.tensor_scalar_add(
                            out=u_t[:, cj, ch * 512:(ch + 1) * 512], in0=ps_u,
                            scalar1=gk_sb[:, cj:cj + 1],
                        )

                # --- sT = h^T(j) @ u ; eT = exp(sc * sT) ---
                eT_t = big.tile([128, TT, HW], f32, name="eT_t")
                for jt in range(TT):
                    ps_s = ps_score.tile([128, CH, 512], f32, name="ps_s")
                    for ch in range(CH):
                        for cj in range(CT):
                            nc.tensor.matmul(
                                ps_s[:, ch, :], h_t[:, cj, jt * 128:(jt + 1) * 128],
                                u_t[:, cj, ch * 512:(ch + 1) * 512],
                                start=(cj == 0), stop=(cj == CT - 1),
                            )
                    for ch in range(CH):
                        nc.scalar.activation(
                            out=eT_t[:, jt, ch * 512:(ch + 1) * 512], in_=ps_s[:, ch, :],
                            func=AF.Exp, scale=SC,
                        )

                # --- Z = ones^T @ eT (broadcast over partitions), invZ ---
                invZ_t = big.tile([128, HW], f32, name="invZ_t")
                for ch in range(CH):
                    ps_z = ps_acc.tile([128, 512], f32, tag="acc", name="ps_z")
                    for jt in range(TT):
                        nc.tensor.matmul(
                            ps_z, ones_sb, eT_t[:, jt, ch * 512:(ch + 1) * 512],
                            start=(jt == 0), stop=(jt == TT - 1),
                        )
                    nc.vector.reciprocal(out=invZ_t[:, ch * 512:(ch + 1) * 512], in_=ps_z)

                # --- oT = (v^T @ eT) * invZ + bv  [c, query i] ---
                oT_t = big.tile([128, CT, HW], f32, name="oT_t")
                for c in range(CT):
                    for ch in range(CH):
                        ps_o = ps_acc.tile([128, 512], f32, tag="acc", name="ps_o")
                        for jt in range(TT):
                            nc.tensor.matmul(
                                ps_o, v_t[:, jt, c * 128:(c + 1) * 128],
                                eT_t[:, jt, ch * 512:(ch + 1) * 512],
                                start=(jt == 0), stop=(jt == TT - 1),
                            )
                        sl = slice(ch * 512, (ch + 1) * 512)
                        nc.vector.tensor_mul(
                            out=oT_t[:, c, sl], in0=ps_o, in1=invZ_t[:, sl]
                        )
                        nc.vector.tensor_scalar_add(
                            out=oT_t[:, c, sl], in0=oT_t[:, c, sl],
                            scalar1=vecs[:, c, 3:4],
                        )

                # --- fT = woT^T @ oT ; y = x + bo + fT ---
                for cp in range(CT):
                    y_t = yout.tile([128, HW], f32, name="y_t")
                    for ch in range(CH):
                        ps_f = ps_acc.tile([128, 512], f32, tag="acc", name="ps_f")
                        for c in range(CT):
                            nc.tensor.matmul(
                                ps_f, woT_t[:, c, cp * 128:(cp + 1) * 128],
                                oT_t[:, c, ch * 512:(ch + 1) * 512],
                                start=(c == 0), stop=(c == CT - 1),
                            )
                        sl = slice(ch * 512, (ch + 1) * 512)
                        nc.vector.tensor_add(
                            out=y_t[:, sl], in0=ps_f, in1=x_t[:, cp, sl]
                        )
                    nc.sync.dma_start(
                        out=y_d[b, cp * 128:(cp + 1) * 128, :], in_=y_t
                    )
    return nc


def _const_inputs():
    bd = np.zeros((128, 128), np.float32)
    for g in range(128 // G):
        bd[g * G:(g + 1) * G, g * G:(g + 1) * G] = 1.0 / G
    return {
        "bd16": bd,
        "ident": np.eye(128, dtype=np.float32),
    }


def run_hw(inputs, trace=False):
    from concourse import bacc
    from concourse.bass_utils import run_bass_kernel_spmd

    x = np.ascontiguousarray(np.asarray(inputs["x"], dtype=np.float32)).reshape(B, C, HW)
    base = dict(_const_inputs())
    for k in ("wq", "wk", "wv", "wo"):
        base[k] = np.ascontiguousarray(np.asarray(inputs[k], dtype=np.float32))
    for k in ("norm_w", "norm_b", "bq", "bk", "bv", "bo"):
        base[k] = np.ascontiguousarray(
            np.asarray(inputs[k], dtype=np.float32).reshape(C, 1)
        )

    nc = bacc.Bacc("TRN2", target_bir_lowering=False)
    build_program(nc)
    nc.finalize()

    in_maps = [
        {**base, "x": np.ascontiguousarray(x[i * BL:(i + 1) * BL])}
        for i in range(NCORES)
    ]
    res = run_bass_kernel_spmd(nc, in_maps, list(range(NCORES)), trace=trace)
    y = np.concatenate([res.results[i]["y"] for i in range(NCORES)], axis=0)
    return y.reshape(B, C, H, W_SP).astype(np.float32), res


def kernel(**inputs):
    y, _ = run_hw(inputs, trace=False)
    return y


# revision 11
# speedup vs baseline: 3.8348x; 3.8348x over previous
"""AttentionBlock (GroupNorm + single-head self-attention + residual) on 8 trn2 cores.

Data-parallel over batch: B=16 -> 2 batch elements per core. Per batch element
(C=512 channels, T=H*W=1024 tokens), everything is kept in channel-major
[C, T] layouts so the whole chain needs zero activation transposes:

  h  = groupnorm(x)                 [C, T]   (bn_stats per channel + block-diag
                                              matmul for cross-partition group agg)
  W  = wq^T @ wk                    [C, C]   (once per core; uses native [O,C] layout)
  u  = W^T @ h  (+ gk := wk^T bq)   [C, T]
  sT = h^T(j) @ u                   [T, T]   scores transposed: [key j, query i]
  eT = exp(sT * C^-1/2)             [T, T]   unnormalized softmax numerator
  Z  = ones^T @ eT                  per-query sums, broadcast to 128 partitions
  oT = (v^T @ eT) * (1/Z) + bv      [C, T]   v = h^T @ wv^T
  fT = wo^T' @ oT                   [C, T]
  y  = x + fT + bo
"""

import numpy as np

B, C, HW = 16, 512, 1024
H = W_SP = 32
G = 16  # channels per group (num_groups=32)
NCORES = 8
BL = B // NCORES  # 2 batch elements per core
CT = C // 128  # 4 channel tiles
TT = HW // 128  # 8 token tiles
CH = HW // 512  # 2 free-dim chunks of 512
EPS = 1e-5
SC = float(C) ** -0.5


def build_program(nc, reps=1, fast=True):
    import concourse.bass as bass
    import concourse.tile as tile
    from concourse import mybir

    f32 = mybir.dt.float32
    f32r = mybir.dt.float32r
    AF = mybir.ActivationFunctionType
    OP = mybir.AluOpType

    # float32r streams 1 row/cycle on the PE (vs 4 for fp32) for N>=256.
    # Tiles feeding f32r matmuls must be written as f32r by their producer op.
    fdt = f32r if fast else f32

    def mm(out, lhsT, rhs, start, stop):
        nc.tensor.matmul(out, lhsT, rhs, start=start, stop=stop)

    x_d = nc.dram_tensor("x", [BL, C, HW], f32, kind="ExternalInput")
    wq_d = nc.dram_tensor("wq", [C, C], f32, kind="ExternalInput")
    wk_d = nc.dram_tensor("wk", [C, C], f32, kind="ExternalInput")
    wv_d = nc.dram_tensor("wv", [C, C], f32, kind="ExternalInput")
    wo_d = nc.dram_tensor("wo", [C, C], f32, kind="ExternalInput")
    nw_d = nc.dram_tensor("norm_w", [C, 1], f32, kind="ExternalInput")
    nb_d = nc.dram_tensor("norm_b", [C, 1], f32, kind="ExternalInput")
    bq_d = nc.dram_tensor("bq", [C, 1], f32, kind="ExternalInput")
    bk_d = nc.dram_tensor("bk", [C, 1], f32, kind="ExternalInput")
    bv_d = nc.dram_tensor("bv", [C, 1], f32, kind="ExternalInput")
    bo_d = nc.dram_tensor("bo", [C, 1], f32, kind="ExternalInput")
    bd_d = nc.dram_tensor("bd16", [128, 128], f32, kind="ExternalInput")
    id_d = nc.dram_tensor("ident", [128, 128], f32, kind="ExternalInput")
    y_d = nc.dram_tensor("y", [BL, C, HW], f32, kind="ExternalOutput")

    with tile.TileContext(nc) as tc:
        with (
            tc.tile_pool(name="persist", bufs=1) as persist,
            tc.tile_pool(name="wtmp", bufs=1) as wtmp,
            tc.tile_pool(name="xin", bufs=2) as xin,
            tc.tile_pool(name="big", bufs=1) as big,
            tc.tile_pool(name="yout", bufs=3) as yout,
            tc.tile_pool(name="small", bufs=2) as small,
            tc.tile_pool(name="ps_score", bufs=2, space="PSUM") as ps_score,
            tc.tile_pool(name="ps_acc", bufs=3, space="PSUM") as ps_acc,
        ):
            # ---------------- startup: weights + constants ----------------
            wq_sb = wtmp.tile([128, CT, C], f32)
            wk_sb = wtmp.tile([128, CT, C], f32)
            wv_sb = wtmp.tile([128, CT, C], f32)
            wo_sb = wtmp.tile([128, CT, C], f32)
            for oi in range(CT):
                nc.sync.dma_start(out=wq_sb[:, oi, :], in_=wq_d[oi * 128:(oi + 1) * 128, :])
                nc.sync.dma_start(out=wk_sb[:, oi, :], in_=wk_d[oi * 128:(oi + 1) * 128, :])
                nc.sync.dma_start(out=wv_sb[:, oi, :], in_=wv_d[oi * 128:(oi + 1) * 128, :])
                nc.sync.dma_start(out=wo_sb[:, oi, :], in_=wo_d[oi * 128:(oi + 1) * 128, :])

            vecs = persist.tile([128, CT, 6], f32)  # norm_w, norm_b, bq, bv, bo, (spare)
            for ci in range(CT):
                sl = slice(ci * 128, (ci + 1) * 128)
                nc.sync.dma_start(out=vecs[:, ci, 0:1], in_=nw_d[sl, :])
                nc.sync.dma_start(out=vecs[:, ci, 1:2], in_=nb_d[sl, :])
                nc.sync.dma_start(out=vecs[:, ci, 2:3], in_=bq_d[sl, :])
                nc.sync.dma_start(out=vecs[:, ci, 3:4], in_=bv_d[sl, :])
                nc.sync.dma_start(out=vecs[:, ci, 4:5], in_=bo_d[sl, :])
                nc.sync.dma_start(out=vecs[:, ci, 5:6], in_=bk_d[sl, :])
            bd_sb = persist.tile([128, 128], f32)
            nc.sync.dma_start(out=bd_sb, in_=bd_d[:, :])
            id_sb = persist.tile([128, 128], f32)
            nc.sync.dma_start(out=id_sb, in_=id_d[:, :])
            ones_f = persist.tile([128, 128], f32)
            nc.vector.memset(ones_f, 1.0)
            ones_sb = persist.tile([128, 128], fdt)
            nc.vector.tensor_copy(out=ones_sb, in_=ones_f)
            eps_sb = persist.tile([128, 1], f32)
            nc.vector.memset(eps_sb, EPS)

            # W = wq^T @ wk  [ci, cj]  (contraction over output channel o)
            W_t = persist.tile([128, CT, C], fdt)
            for ci in range(CT):
                ps_w = ps_acc.tile([128, 512], f32, tag="acc", name="ps_w")
                for oi in range(CT):
                    nc.tensor.matmul(
                        ps_w, wq_sb[:, oi, ci * 128:(ci + 1) * 128], wk_sb[:, oi, :],
                        start=(oi == 0), stop=(oi == CT - 1),
                    )
                nc.scalar.copy(out=W_t[:, ci, :], in_=ps_w)

            # wvT / woT via PE transpose of 128x128 blocks
            wvT_t = persist.tile([128, CT, C], fdt)
            woT_t = persist.tile([128, CT, C], fdt)
            for src, dst in ((wv_sb, wvT_t), (wo_sb, woT_t)):
                for oi in range(CT):
                    for ci in range(CT):
                        ps_t = ps_acc.tile([128, 128], f32, tag="acc", name="ps_t")
                        nc.tensor.transpose(
                            ps_t, src[:, oi, ci * 128:(ci + 1) * 128], id_sb
                        )
                        nc.scalar.copy(
                            out=dst[:, ci, oi * 128:(oi + 1) * 128], in_=ps_t
                        )

            # gk = wk^T @ bq  (per-key score bias correction), [C] as [128, CT]
            gk_sb = persist.tile([128, CT], f32)
            for ci in range(CT):
                ps_g = ps_acc.tile([128, 1], f32, tag="acc", name="ps_g")
                for oi in range(CT):
                    nc.tensor.matmul(
                        ps_g, wk_sb[:, oi, ci * 128:(ci + 1) * 128], vecs[:, oi, 2:3],
                        start=(oi == 0), stop=(oi == CT - 1),
                    )
                nc.scalar.copy(out=gk_sb[:, ci:ci + 1], in_=ps_g)

            # ---------------- per batch element ----------------
            for b in [b for _ in range(reps) for b in range(BL)]:
                x_t = xin.tile([128, CT, HW], f32, name="x_t")
                for ci in range(CT):
                    nc.sync.dma_start(
                        out=x_t[:, ci, :], in_=x_d[b, ci * 128:(ci + 1) * 128, :]
                    )

                # --- group norm ---
                h_t = big.tile([128, CT, HW], fdt, name="h_t")
                for ci in range(CT):
                    stats = small.tile([128, 2, 6], f32, name="stats")
                    for s in range(2):
                        nc.vector.bn_stats(
                            out=stats[:, s, :], in_=x_t[:, ci, s * 512:(s + 1) * 512]
                        )
                    mv = small.tile([128, 2], f32, name="mv")
                    nc.vector.bn_aggr(out=mv, in_=stats)
                    st2 = small.tile([128, 2], f32, name="st2")
                    nc.vector.tensor_copy(out=st2[:, 0:1], in_=mv[:, 0:1])
                    nc.vector.tensor_mul(out=st2[:, 1:2], in0=mv[:, 0:1], in1=mv[:, 0:1])
                    nc.vector.tensor_add(out=st2[:, 1:2], in0=st2[:, 1:2], in1=mv[:, 1:2])
                    ps_st = ps_acc.tile([128, 2], f32, tag="acc", name="ps_st")
                    nc.tensor.matmul(ps_st, bd_sb, st2, start=True, stop=True)
                    mug = small.tile([128, 1], f32, name="mug")
                    nc.vector.tensor_copy(out=mug, in_=ps_st[:, 0:1])
                    tv = small.tile([128, 1], f32, name="tv")
                    nc.vector.tensor_mul(out=tv, in0=mug, in1=mug)
                    nc.vector.tensor_sub(out=tv, in0=ps_st[:, 1:2], in1=tv)
                    nc.scalar.activation(out=tv, in_=tv, func=AF.Sqrt, bias=eps_sb, scale=1.0)
                    nc.vector.reciprocal(out=tv, in_=tv)
                    sc_c = small.tile([128, 1], f32, name="sc_c")
                    nc.vector.tensor_mul(out=sc_c, in0=tv, in1=vecs[:, ci, 0:1])
                    bi_c = small.tile([128, 1], f32, name="bi_c")
                    nc.vector.tensor_mul(out=bi_c, in0=mug, in1=sc_c)
                    nc.vector.tensor_sub(out=bi_c, in0=vecs[:, ci, 1:2], in1=bi_c)
                    nc.vector.tensor_scalar(
                        out=h_t[:, ci, :], in0=x_t[:, ci, :],
                        scalar1=sc_c, scalar2=bi_c, op0=OP.mult, op1=OP.add,
                    )
                    # x_t becomes (x + bo) for the final residual
                    nc.scalar.activation(
                        out=x_t[:, ci, :], in_=x_t[:, ci, :], func=AF.Identity,
                        bias=vecs[:, ci, 4:5], scale=1.0,
                    )

                # --- v = h^T @ wv^T  [token, c_out] ---
                v_t = big.tile([128, TT, 512], fdt, name="v_t")
                for tt in range(TT):
                    ps_v = ps_acc.tile([128, 512], f32, tag="acc", name="ps_v")
                    for ci in range(CT):
                        mm(
                            ps_v, h_t[:, ci, tt * 128:(tt + 1) * 128], wvT_t[:, ci, :],
                            start=(ci == 0), stop=(ci == CT - 1),
                        )
                    nc.scalar.copy(out=v_t[:, tt, :], in_=ps_v)

                # --- u = W^T @ h (+gk)  [cj, query i] ---
                u_t = big.tile([128, CT, HW], fdt, name="u_t")
                for cj in range(CT):
                    for ch in range(CH):
                        ps_u = ps_acc.tile([128, 512], f32, tag="acc", name="ps_u")
                        for ci in range(CT):
                            mm(
                                ps_u, W_t[:, ci, cj * 128:(cj + 1) * 128],
                                h_t[:, ci, ch * 512:(ch + 1) * 512],
                                start=(ci == 0), stop=(ci == CT - 1),
                            )
                        nc.vector.tensor_scalar_add(
                            out=u_t[:, cj, ch * 512:(ch + 1) * 512], in0=ps_u,
                            scalar1=gk_sb[:, cj:cj + 1],
                        )

                # --- sT = h^T(j) @ u ; eT = exp(sc * sT) ---
                eT_t = big.tile([128, TT, HW], fdt, name="eT_t")
                for jt in range(TT):
                    ps_s = ps_score.tile([128, CH, 512], f32, name="ps_s")
                    for ch in range(CH):
                        for cj in range(CT):
                            mm(
                                ps_s[:, ch, :], h_t[:, cj, jt * 128:(jt + 1) * 128],
                                u_t[:, cj, ch * 512:(ch + 1) * 512],
                                start=(cj == 0), stop=(cj == CT - 1),
                            )
                    for ch in range(CH):
                        nc.scalar.activation(
                            out=eT_t[:, jt, ch * 512:(ch + 1) * 512], in_=ps_s[:, ch, :],
                            func=AF.Exp, scale=SC,
                        )

                # --- Z = ones^T @ eT (broadcast over partitions), invZ ---
                invZ_t = big.tile([128, HW], f32, name="invZ_t")
                for ch in range(CH):
                    ps_z = ps_acc.tile([128, 512], f32, tag="acc", name="ps_z")
                    for jt in range(TT):
                        mm(
                            ps_z, ones_sb, eT_t[:, jt, ch * 512:(ch + 1) * 512],
                            start=(jt == 0), stop=(jt == TT - 1),
                        )
                    nc.vector.reciprocal(out=invZ_t[:, ch * 512:(ch + 1) * 512], in_=ps_z)

                # --- oT = (v^T @ eT) * invZ + bv  [c, query i] ---
                oT_t = big.tile([128, CT, HW], fdt, name="oT_t")
                for c in range(CT):
                    for ch in range(CH):
                        ps_o = ps_acc.tile([128, 512], f32, tag="acc", name="ps_o")
                        for jt in range(TT):
                            mm(
                                ps_o, v_t[:, jt, c * 128:(c + 1) * 128],
                                eT_t[:, jt, ch * 512:(ch + 1) * 512],
                                start=(jt == 0), stop=(jt == TT - 1),
                            )
                        sl = slice(ch * 512, (ch + 1) * 512)
                        nc.vector.tensor_mul(
                            out=oT_t[:, c, sl], in0=ps_o, in1=invZ_t[:, sl]
                        )
                        nc.vector.tensor_scalar_add(
                            out=oT_t[:, c, sl], in0=oT_t[:, c, sl],
                            scalar1=vecs[:, c, 3:4],
                        )

                # --- fT = woT^T @ oT ; y = x + bo + fT ---
                for cp in range(CT):
                    y_t = yout.tile([128, HW], f32, name="y_t")
                    for ch in range(CH):
                        ps_f = ps_acc.tile([128, 512], f32, tag="acc", name="ps_f")
                        for c in range(CT):
                            mm(
                                ps_f, woT_t[:, c, cp * 128:(cp + 1) * 128],
                                oT_t[:, c, ch * 512:(ch + 1) * 512],
                                start=(c == 0), stop=(c == CT - 1),
                            )
                        sl = slice(ch * 512, (ch + 1) * 512)
                        nc.vector.tensor_add(
                            out=y_t[:, sl], in0=ps_f, in1=x_t[:, cp, sl]
                        )
                    nc.sync.dma_start(
                        out=y_d[b, cp * 128:(cp + 1) * 128, :], in_=y_t
                    )
    return nc


def _const_inputs():
    bd = np.zeros((128, 128), np.float32)
    for g in range(128 // G):
        bd[g * G:(g + 1) * G, g * G:(g + 1) * G] = 1.0 / G
    return {
        "bd16": bd,
        "ident": np.eye(128, dtype=np.float32),
    }


def run_hw(inputs, trace=False):
    from concourse import bacc
    from concourse.bass_utils import run_bass_kernel_spmd

    x = np.ascontiguousarray(np.asarray(inputs["x"], dtype=np.float32)).reshape(B, C, HW)
    base = dict(_const_inputs())
    for k in ("wq", "wk", "wv", "wo"):
        base[k] = np.ascontiguousarray(np.asarray(inputs[k], dtype=np.float32))
    for k in ("norm_w", "norm_b", "bq", "bk", "bv", "bo"):
        base[k] = np.ascontiguousarray(
            np.asarray(inputs[k], dtype=np.float32).reshape(C, 1)
        )

    nc = bacc.Bacc("TRN2", target_bir_lowering=False)
    build_program(nc)
    nc.finalize()

    in_maps = [
        {**base, "x": np.ascontiguousarray(x[i * BL:(i + 1) * BL])}
        for i in range(NCORES)
    ]
    res = run_bass_kernel_spmd(nc, in_maps, list(range(NCORES)), trace=trace)
    y = np.concatenate([res.results[i]["y"] for i in range(NCORES)], axis=0)
    return y.reshape(B, C, H, W_SP).astype(np.float32), res


def kernel(**inputs):
    y, _ = run_hw(inputs, trace=False)
    return y


# revision 17
# speedup vs baseline: 4.4186x; 1.1522x over previous
"""AttentionBlock (GroupNorm + single-head self-attention + residual) on 8 trn2 cores.

Data-parallel over batch: B=16 -> 2 batch elements per core. Per batch element
(C=512 channels, T=H*W=1024 tokens), everything is kept in channel-major
[C, T] layouts so the whole chain needs zero activation transposes:

  h  = groupnorm(x)                 [C, T]   (bn_stats per channel + block-diag
                                              matmul for cross-partition group agg)
  W  = wq^T @ wk                    [C, C]   (once per core; uses native [O,C] layout)
  u  = W^T @ h  (+ gk := wk^T bq)   [C, T]
  sT = h^T(j) @ u                   [T, T]   scores transposed: [key j, query i]
  eT = exp(sT * C^-1/2)             [T, T]   unnormalized softmax numerator
  Z  = ones^T @ eT                  per-query sums, broadcast to 128 partitions
  oT = (v^T @ eT) * (1/Z) + bv      [C, T]   v = h^T @ wv^T
  fT = wo^T' @ oT                   [C, T]
  y  = x + fT + bo
"""

import numpy as np

B, C, HW = 16, 512, 1024
H = W_SP = 32
G = 16  # channels per group (num_groups=32)
NCORES = 8
BL = B // NCORES  # 2 batch elements per core
CT = C // 128  # 4 channel tiles
TT = HW // 128  # 8 token tiles
CH = HW // 512  # 2 free-dim chunks of 512
EPS = 1e-5
SC = float(C) ** -0.5


def build_program(nc, reps=1, fast=True):
    import concourse.bass as bass
    import concourse.tile as tile
    from concourse import mybir

    f32 = mybir.dt.float32
    f32r = mybir.dt.float32r
    AF = mybir.ActivationFunctionType
    OP = mybir.AluOpType

    # float32r streams 1 row/cycle on the PE (vs 4 for fp32) for N>=256.
    # Tiles feeding f32r matmuls must be written as f32r by their producer op.
    fdt = f32r if fast else f32

    def mm(out, lhsT, rhs, start, stop):
        nc.tensor.matmul(out, lhsT, rhs, start=start, stop=stop)

    x_d = nc.dram_tensor("x", [BL, C, HW], f32, kind="ExternalInput")
    W_d = nc.dram_tensor("Wqk", [C, C], f32, kind="ExternalInput")
    wvT_d = nc.dram_tensor("wvT", [C, C], f32, kind="ExternalInput")
    woT_d = nc.dram_tensor("woT", [C, C], f32, kind="ExternalInput")
    # vecs columns: 0=norm_w 1=norm_b 2=gk(=wk^T bq) 3=wob(=wo bv + bo)
    vec_d = nc.dram_tensor("vecs", [C, 4], f32, kind="ExternalInput")
    bd_d = nc.dram_tensor("bd16", [128, 128], f32, kind="ExternalInput")
    y_d = nc.dram_tensor("y", [BL, C, HW], f32, kind="ExternalOutput")

    with tile.TileContext(nc) as tc:
        with (
            tc.tile_pool(name="persist", bufs=1) as persist,
            tc.tile_pool(name="wtmp", bufs=1) as wtmp,
            tc.tile_pool(name="xin", bufs=2) as xin,
            tc.tile_pool(name="big", bufs=1) as big,
            tc.tile_pool(name="yout", bufs=3) as yout,
            tc.tile_pool(name="small", bufs=2) as small,
            tc.tile_pool(name="ps_score", bufs=2, space="PSUM") as ps_score,
            tc.tile_pool(name="ps_acc", bufs=4, space="PSUM") as ps_acc,
        ):
            # ---------------- startup: weights + constants ----------------
            # All weight algebra (W=wq^T wk, wv^T, wo^T, gk, wob) is done on
            # the host; the device only loads + rounds to f32r. DMAs go on the
            # otherwise-idle gpsimd DGE so x (on SP) lands first.
            bd_sb = persist.tile([128, 128], f32)
            nc.gpsimd.dma_start(out=bd_sb, in_=bd_d[:, :])
            vecs = persist.tile([128, CT, 4], f32)
            for ci in range(CT):
                nc.gpsimd.dma_start(
                    out=vecs[:, ci, :], in_=vec_d[ci * 128:(ci + 1) * 128, :]
                )
            Wf = wtmp.tile([128, CT, C], f32)
            vTf = wtmp.tile([128, CT, C], f32)
            oTf = wtmp.tile([128, CT, C], f32)
            for ci in range(CT):
                sl = slice(ci * 128, (ci + 1) * 128)
                nc.gpsimd.dma_start(out=vTf[:, ci, :], in_=wvT_d[sl, :])
                nc.gpsimd.dma_start(out=Wf[:, ci, :], in_=W_d[sl, :])
                nc.gpsimd.dma_start(out=oTf[:, ci, :], in_=woT_d[sl, :])
            eps_sb = persist.tile([128, 1], f32)
            nc.vector.memset(eps_sb, EPS)
            ones_f = persist.tile([128, 128], f32)
            nc.vector.memset(ones_f, 1.0)
            ones_sb = persist.tile([128, 128], fdt)
            nc.vector.tensor_copy(out=ones_sb, in_=ones_f)

            # round to f32r (structural requirement for f32r matmul operands)
            W_t = persist.tile([128, CT, C], fdt)
            wvT_t = persist.tile([128, CT, C], fdt)
            woT_t = persist.tile([128, CT, C], fdt)
            for ci in range(CT):
                nc.vector.tensor_copy(out=wvT_t[:, ci, :], in_=vTf[:, ci, :])
                nc.vector.tensor_copy(out=W_t[:, ci, :], in_=Wf[:, ci, :])
                nc.gpsimd.tensor_copy(out=woT_t[:, ci, :], in_=oTf[:, ci, :])

            # ---------------- per batch element ----------------
            for b in [b for _ in range(reps) for b in range(BL)]:
                x_t = xin.tile([128, CT, HW], f32, name="x_t")
                for ci in range(CT):
                    nc.sync.dma_start(
                        out=x_t[:, ci, :], in_=x_d[b, ci * 128:(ci + 1) * 128, :]
                    )

                # --- group norm ---
                h_t = big.tile([128, CT, HW], fdt, name="h_t")
                for ci in range(CT):
                    stats = small.tile([128, 2, 6], f32, name="stats")
                    for s in range(2):
                        nc.vector.bn_stats(
                            out=stats[:, s, :], in_=x_t[:, ci, s * 512:(s + 1) * 512]
                        )
                    mv = small.tile([128, 2], f32, name="mv")
                    nc.vector.bn_aggr(out=mv, in_=stats)
                    st2 = small.tile([128, 2], f32, name="st2")
                    nc.vector.tensor_copy(out=st2[:, 0:1], in_=mv[:, 0:1])
                    nc.vector.tensor_mul(out=st2[:, 1:2], in0=mv[:, 0:1], in1=mv[:, 0:1])
                    nc.vector.tensor_add(out=st2[:, 1:2], in0=st2[:, 1:2], in1=mv[:, 1:2])
                    ps_st = ps_acc.tile([128, 2], f32, tag="acc", name="ps_st")
                    nc.tensor.matmul(ps_st, bd_sb, st2, start=True, stop=True)
                    mug = small.tile([128, 1], f32, name="mug")
                    nc.vector.tensor_copy(out=mug, in_=ps_st[:, 0:1])
                    tv = small.tile([128, 1], f32, name="tv")
                    nc.vector.tensor_mul(out=tv, in0=mug, in1=mug)
                    nc.vector.tensor_sub(out=tv, in0=ps_st[:, 1:2], in1=tv)
                    nc.scalar.activation(out=tv, in_=tv, func=AF.Sqrt, bias=eps_sb, scale=1.0)
                    nc.vector.reciprocal(out=tv, in_=tv)
                    sc_c = small.tile([128, 1], f32, name="sc_c")
                    nc.vector.tensor_mul(out=sc_c, in0=tv, in1=vecs[:, ci, 0:1])
                    bi_c = small.tile([128, 1], f32, name="bi_c")
                    nc.vector.tensor_mul(out=bi_c, in0=mug, in1=sc_c)
                    nc.vector.tensor_sub(out=bi_c, in0=vecs[:, ci, 1:2], in1=bi_c)
                    nc.gpsimd.tensor_scalar(
                        out=h_t[:, ci, :], in0=x_t[:, ci, :],
                        scalar1=sc_c, scalar2=bi_c, op0=OP.mult, op1=OP.add,
                    )
                    # x_t becomes (x + bo) for the final residual
                    nc.scalar.activation(
                        out=x_t[:, ci, :], in_=x_t[:, ci, :], func=AF.Identity,
                        bias=vecs[:, ci, 3:4], scale=1.0,
                    )

                # --- v = h^T @ wv^T  [token, c_out] ---
                v_t = big.tile([128, TT, 512], fdt, name="v_t")
                for tt in range(TT):
                    ps_v = ps_acc.tile([128, 512], f32, tag="acc", name="ps_v")
                    for ci in range(CT):
                        mm(
                            ps_v, h_t[:, ci, tt * 128:(tt + 1) * 128], wvT_t[:, ci, :],
                            start=(ci == 0), stop=(ci == CT - 1),
                        )
                    if tt % 2 == 0:
                        nc.scalar.copy(out=v_t[:, tt, :], in_=ps_v)
                    else:
                        nc.vector.tensor_copy(out=v_t[:, tt, :], in_=ps_v)

                # --- u = W^T @ h (+gk)  [cj, query i] ---
                u_t = big.tile([128, CT, HW], fdt, name="u_t")
                for cj in range(CT):
                    for ch in range(CH):
                        ps_u = ps_acc.tile([128, 512], f32, tag="acc", name="ps_u")
                        for ci in range(CT):
                            mm(
                                ps_u, W_t[:, ci, cj * 128:(cj + 1) * 128],
                                h_t[:, ci, ch * 512:(ch + 1) * 512],
                                start=(ci == 0), stop=(ci == CT - 1),
                            )
                        if (cj + ch) % 2 == 0:
                            nc.vector.tensor_scalar_add(
                                out=u_t[:, cj, ch * 512:(ch + 1) * 512], in0=ps_u,
                                scalar1=vecs[:, cj, 2:3],
                            )
                        else:
                            nc.scalar.activation(
                                out=u_t[:, cj, ch * 512:(ch + 1) * 512], in_=ps_u,
                                func=AF.Identity, bias=vecs[:, cj, 2:3], scale=1.0,
                            )

                # --- sT = h^T(j) @ u ; eT = exp(sc * sT) ---
                eT_t = big.tile([128, TT, HW], fdt, name="eT_t")
                for jt in range(TT):
                    ps_s = ps_score.tile([128, CH, 512], f32, name="ps_s")
                    for ch in range(CH):
                        for cj in range(CT):
                            mm(
                                ps_s[:, ch, :], h_t[:, cj, jt * 128:(jt + 1) * 128],
                                u_t[:, cj, ch * 512:(ch + 1) * 512],
                                start=(cj == 0), stop=(cj == CT - 1),
                            )
                    for ch in range(CH):
                        nc.scalar.activation(
                            out=eT_t[:, jt, ch * 512:(ch + 1) * 512], in_=ps_s[:, ch, :],
                            func=AF.Exp, scale=SC,
                        )

                # --- Z = ones^T @ eT (broadcast over partitions), invZ ---
                invZ_t = big.tile([128, HW], f32, name="invZ_t")
                for ch in range(CH):
                    ps_z = ps_acc.tile([128, 512], f32, tag="acc", name="ps_z")
                    for jt in range(TT):
                        mm(
                            ps_z, ones_sb, eT_t[:, jt, ch * 512:(ch + 1) * 512],
                            start=(jt == 0), stop=(jt == TT - 1),
                        )
                    nc.vector.reciprocal(out=invZ_t[:, ch * 512:(ch + 1) * 512], in_=ps_z)

                # --- oT = (v^T @ eT) * invZ + bv  [c, query i] ---
                oT_t = big.tile([128, CT, HW], fdt, name="oT_t")
                for c in range(CT):
                    for ch in range(CH):
                        ps_o = ps_acc.tile([128, 512], f32, tag="acc", name="ps_o")
                        for jt in range(TT):
                            mm(
                                ps_o, v_t[:, jt, c * 128:(c + 1) * 128],
                                eT_t[:, jt, ch * 512:(ch + 1) * 512],
                                start=(jt == 0), stop=(jt == TT - 1),
                            )
                        sl = slice(ch * 512, (ch + 1) * 512)
                        nc.vector.tensor_mul(
                            out=oT_t[:, c, sl], in0=ps_o, in1=invZ_t[:, sl]
                        )

                # --- fT = woT^T @ oT ; y = x + bo + fT ---
                for cp in range(CT):
                    y_t = yout.tile([128, HW], f32, name="y_t")
                    for ch in range(CH):
                        ps_f = ps_acc.tile([128, 512], f32, tag="acc", name="ps_f")
                        for c in range(CT):
                            mm(
                                ps_f, woT_t[:, c, cp * 128:(cp + 1) * 128],
                                oT_t[:, c, ch * 512:(ch + 1) * 512],
                                start=(c == 0), stop=(c == CT - 1),
                            )
                        sl = slice(ch * 512, (ch + 1) * 512)
                        nc.vector.tensor_add(
                            out=y_t[:, sl], in0=ps_f, in1=x_t[:, cp, sl]
                        )
                    nc.sync.dma_start(
                        out=y_d[b, cp * 128:(cp + 1) * 128, :], in_=y_t
                    )
    return nc


def _const_inputs():
    bd = np.zeros((128, 128), np.float32)
    for g in range(128 // G):
        bd[g * G:(g + 1) * G, g * G:(g + 1) * G] = 1.0 / G
    return {"bd16": bd}


def prep_inputs(inputs):
    x = np.ascontiguousarray(np.asarray(inputs["x"], dtype=np.float32)).reshape(B, C, HW)
    wq = np.asarray(inputs["wq"], dtype=np.float32)
    wk = np.asarray(inputs["wk"], dtype=np.float32)
    wv = np.asarray(inputs["wv"], dtype=np.float32)
    wo = np.asarray(inputs["wo"], dtype=np.float32)
    bq = np.asarray(inputs["bq"], dtype=np.float32).reshape(C)
    bv = np.asarray(inputs["bv"], dtype=np.float32).reshape(C)
    bo = np.asarray(inputs["bo"], dtype=np.float32).reshape(C)
    nw = np.asarray(inputs["norm_w"], dtype=np.float32).reshape(C)
    nb = np.asarray(inputs["norm_b"], dtype=np.float32).reshape(C)
    base = dict(_const_inputs())
    base["Wqk"] = np.ascontiguousarray(wq.T @ wk)
    base["wvT"] = np.ascontiguousarray(wv.T)
    base["woT"] = np.ascontiguousarray(wo.T)
    gk = wk.T @ bq
    wob = wo @ bv + bo
    base["vecs"] = np.ascontiguousarray(np.stack([nw, nb, gk, wob], axis=1))
    return base, x


def run_hw(inputs, trace=False):
    from concourse import bacc
    from concourse.bass_utils import run_bass_kernel_spmd

    base, x = prep_inputs(inputs)

    nc = bacc.Bacc("TRN2", target_bir_lowering=False)
    build_program(nc)
    nc.finalize()

    in_maps = [
        {**base, "x": np.ascontiguousarray(x[i * BL:(i + 1) * BL])}
        for i in range(NCORES)
    ]
    res = run_bass_kernel_spmd(nc, in_maps, list(range(NCORES)), trace=trace)
    y = np.concatenate([res.results[i]["y"] for i in range(NCORES)], axis=0)
    return y.reshape(B, C, H, W_SP).astype(np.float32), res


def kernel(**inputs):
    y, _ = run_hw(inputs, trace=False)
    return y


# revision 18
# speedup vs baseline: 4.5845x; 1.0375x over previous
"""AttentionBlock (GroupNorm + single-head self-attention + residual) on 8 trn2 cores.

Data-parallel over batch: B=16 -> 2 batch elements per core. Per batch element
(C=512 channels, T=H*W=1024 tokens), everything is kept in channel-major
[C, T] layouts so the whole chain needs zero activation transposes:

  h  = groupnorm(x)                 [C, T]   (bn_stats per channel + block-diag
                                              matmul for cross-partition group agg)
  W  = wq^T @ wk                    [C, C]   (once per core; uses native [O,C] layout)
  u  = W^T @ h  (+ gk := wk^T bq)   [C, T]
  sT = h^T(j) @ u                   [T, T]   scores transposed: [key j, query i]
  eT = exp(sT * C^-1/2)             [T, T]   unnormalized softmax numerator
  Z  = ones^T @ eT                  per-query sums, broadcast to 128 partitions
  oT = (v^T @ eT) * (1/Z) + bv      [C, T]   v = h^T @ wv^T
  fT = wo^T' @ oT                   [C, T]
  y  = x + fT + bo
"""

import numpy as np

B, C, HW = 16, 512, 1024
H = W_SP = 32
G = 16  # channels per group (num_groups=32)
NCORES = 8
BL = B // NCORES  # 2 batch elements per core
CT = C // 128  # 4 channel tiles
TT = HW // 128  # 8 token tiles
CH = HW // 512  # 2 free-dim chunks of 512
EPS = 1e-5
SC = float(C) ** -0.5


def build_program(nc, reps=1, fast=True):
    import concourse.bass as bass
    import concourse.tile as tile
    from concourse import mybir

    f32 = mybir.dt.float32
    f32r = mybir.dt.float32r
    AF = mybir.ActivationFunctionType
    OP = mybir.AluOpType

    # float32r streams 1 row/cycle on the PE (vs 4 for fp32) for N>=256.
    # Tiles feeding f32r matmuls must be written as f32r by their producer op.
    fdt = f32r if fast else f32

    def mm(out, lhsT, rhs, start, stop):
        nc.tensor.matmul(out, lhsT, rhs, start=start, stop=stop)

    x_d = nc.dram_tensor("x", [BL, C, HW], f32, kind="ExternalInput")
    W_d = nc.dram_tensor("Wqk", [C, C], f32, kind="ExternalInput")
    wvT_d = nc.dram_tensor("wvT", [C, C], f32, kind="ExternalInput")
    woT_d = nc.dram_tensor("woT", [C, C], f32, kind="ExternalInput")
    # vecs columns: 0=norm_w 1=norm_b 2=gk(=wk^T bq) 3=wob(=wo bv + bo)
    vec_d = nc.dram_tensor("vecs", [C, 4], f32, kind="ExternalInput")
    bd_d = nc.dram_tensor("bd16", [128, 128], f32, kind="ExternalInput")
    y_d = nc.dram_tensor("y", [BL, C, HW], f32, kind="ExternalOutput")

    with tile.TileContext(nc) as tc:
        with (
            tc.tile_pool(name="persist", bufs=1) as persist,
            tc.tile_pool(name="wtmp", bufs=1) as wtmp,
            tc.tile_pool(name="xin", bufs=2) as xin,
            tc.tile_pool(name="big", bufs=1) as big,
            tc.tile_pool(name="yout", bufs=3) as yout,
            tc.tile_pool(name="small", bufs=2) as small,
            tc.tile_pool(name="ps_score", bufs=2, space="PSUM") as ps_score,
            tc.tile_pool(name="ps_acc", bufs=4, space="PSUM") as ps_acc,
        ):
            # ---------------- startup: weights + constants ----------------
            # All weight algebra (W=wq^T wk, wv^T, wo^T, gk, wob) is done on
            # the host; the device only loads + rounds to f32r. DMAs go on the
            # otherwise-idle gpsimd DGE so x (on SP) lands first.
            bd_sb = persist.tile([128, 128], f32)
            nc.gpsimd.dma_start(out=bd_sb, in_=bd_d[:, :])
            vecs = persist.tile([128, CT, 4], f32)
            for ci in range(CT):
                nc.gpsimd.dma_start(
                    out=vecs[:, ci, :], in_=vec_d[ci * 128:(ci + 1) * 128, :]
                )
            Wf = wtmp.tile([128, CT, C], f32)
            vTf = wtmp.tile([128, CT, C], f32)
            oTf = wtmp.tile([128, CT, C], f32)
            for ci in range(CT):
                sl = slice(ci * 128, (ci + 1) * 128)
                nc.gpsimd.dma_start(out=vTf[:, ci, :], in_=wvT_d[sl, :])
                nc.gpsimd.dma_start(out=Wf[:, ci, :], in_=W_d[sl, :])
                nc.gpsimd.dma_start(out=oTf[:, ci, :], in_=woT_d[sl, :])
            eps_sb = persist.tile([128, 1], f32)
            nc.vector.memset(eps_sb, EPS)
            ones_f = persist.tile([128, 128], f32)
            nc.vector.memset(ones_f, 1.0)
            ones_sb = persist.tile([128, 128], fdt)
            nc.vector.tensor_copy(out=ones_sb, in_=ones_f)

            # round to f32r (structural requirement for f32r matmul operands)
            W_t = persist.tile([128, CT, C], fdt)
            wvT_t = persist.tile([128, CT, C], fdt)
            woT_t = persist.tile([128, CT, C], fdt)
            for ci in range(CT):
                nc.vector.tensor_copy(out=wvT_t[:, ci, :], in_=vTf[:, ci, :])
                nc.vector.tensor_copy(out=W_t[:, ci, :], in_=Wf[:, ci, :])
                nc.gpsimd.tensor_copy(out=woT_t[:, ci, :], in_=oTf[:, ci, :])

            # ---------------- per batch element ----------------
            for b in [b for _ in range(reps) for b in range(BL)]:
                x_t = xin.tile([128, CT, HW], f32, name="x_t")
                for ci in range(CT):
                    for s in range(2):
                        nc.sync.dma_start(
                            out=x_t[:, ci, s * 512:(s + 1) * 512],
                            in_=x_d[b, ci * 128:(ci + 1) * 128, s * 512:(s + 1) * 512],
                        )

                # --- group norm ---
                h_t = big.tile([128, CT, HW], fdt, name="h_t")
                for ci in range(CT):
                    stats = small.tile([128, 2, 6], f32, name="stats")
                    for s in range(2):
                        nc.vector.bn_stats(
                            out=stats[:, s, :], in_=x_t[:, ci, s * 512:(s + 1) * 512]
                        )
                    mv = small.tile([128, 2], f32, name="mv")
                    nc.vector.bn_aggr(out=mv, in_=stats)
                    st2 = small.tile([128, 2], f32, name="st2")
                    nc.vector.tensor_copy(out=st2[:, 0:1], in_=mv[:, 0:1])
                    nc.vector.tensor_mul(out=st2[:, 1:2], in0=mv[:, 0:1], in1=mv[:, 0:1])
                    nc.vector.tensor_add(out=st2[:, 1:2], in0=st2[:, 1:2], in1=mv[:, 1:2])
                    ps_st = ps_acc.tile([128, 2], f32, tag="acc", name="ps_st")
                    nc.tensor.matmul(ps_st, bd_sb, st2, start=True, stop=True)
                    mug = small.tile([128, 1], f32, name="mug")
                    nc.vector.tensor_copy(out=mug, in_=ps_st[:, 0:1])
                    tv = small.tile([128, 1], f32, name="tv")
                    nc.vector.tensor_mul(out=tv, in0=mug, in1=mug)
                    nc.vector.tensor_sub(out=tv, in0=ps_st[:, 1:2], in1=tv)
                    nc.scalar.activation(out=tv, in_=tv, func=AF.Sqrt, bias=eps_sb, scale=1.0)
                    nc.vector.reciprocal(out=tv, in_=tv)
                    sc_c = small.tile([128, 1], f32, name="sc_c")
                    nc.vector.tensor_mul(out=sc_c, in0=tv, in1=vecs[:, ci, 0:1])
                    bi_c = small.tile([128, 1], f32, name="bi_c")
                    nc.vector.tensor_mul(out=bi_c, in0=mug, in1=sc_c)
                    nc.vector.tensor_sub(out=bi_c, in0=vecs[:, ci, 1:2], in1=bi_c)
                    nc.gpsimd.tensor_scalar(
                        out=h_t[:, ci, :], in0=x_t[:, ci, :],
                        scalar1=sc_c, scalar2=bi_c, op0=OP.mult, op1=OP.add,
                    )
                    # x_t becomes (x + bo) for the final residual
                    nc.scalar.activation(
                        out=x_t[:, ci, :], in_=x_t[:, ci, :], func=AF.Identity,
                        bias=vecs[:, ci, 3:4], scale=1.0,
                    )

                # --- v = h^T @ wv^T  [token, c_out] ---
                v_t = big.tile([128, TT, 512], fdt, name="v_t")
                for tt in range(TT):
                    ps_v = ps_acc.tile([128, 512], f32, tag="acc", name="ps_v")
                    for ci in range(CT):
                        mm(
                            ps_v, h_t[:, ci, tt * 128:(tt + 1) * 128], wvT_t[:, ci, :],
                            start=(ci == 0), stop=(ci == CT - 1),
                        )
                    if tt % 2 == 0:
                        nc.scalar.copy(out=v_t[:, tt, :], in_=ps_v)
                    else:
                        nc.vector.tensor_copy(out=v_t[:, tt, :], in_=ps_v)

                # --- u = W^T @ h (+gk)  [cj, query i] ---
                u_t = big.tile([128, CT, HW], fdt, name="u_t")
                for cj in range(CT):
                    for ch in range(CH):
                        ps_u = ps_acc.tile([128, 512], f32, tag="acc", name="ps_u")
                        for ci in range(CT):
                            mm(
                                ps_u, W_t[:, ci, cj * 128:(cj + 1) * 128],
                                h_t[:, ci, ch * 512:(ch + 1) * 512],
                                start=(ci == 0), stop=(ci == CT - 1),
                            )
                        if (cj + ch) % 2 == 0:
                            nc.vector.tensor_scalar_add(
                                out=u_t[:, cj, ch * 512:(ch + 1) * 512], in0=ps_u,
                                scalar1=vecs[:, cj, 2:3],
                            )
                        else:
                            nc.scalar.activation(
                                out=u_t[:, cj, ch * 512:(ch + 1) * 512], in_=ps_u,
                                func=AF.Identity, bias=vecs[:, cj, 2:3], scale=1.0,
                            )

                # --- sT = h^T(j) @ u ; eT = exp(sc * sT) ---
                eT_t = big.tile([128, TT, HW], fdt, name="eT_t")
                for jt in range(TT):
                    ps_s = ps_score.tile([128, CH, 512], f32, name="ps_s")
                    for ch in range(CH):
                        for cj in range(CT):
                            mm(
                                ps_s[:, ch, :], h_t[:, cj, jt * 128:(jt + 1) * 128],
                                u_t[:, cj, ch * 512:(ch + 1) * 512],
                                start=(cj == 0), stop=(cj == CT - 1),
                            )
                    for ch in range(CH):
                        nc.scalar.activation(
                            out=eT_t[:, jt, ch * 512:(ch + 1) * 512], in_=ps_s[:, ch, :],
                            func=AF.Exp, scale=SC,
                        )

                # --- Z = ones^T @ eT (broadcast over partitions), invZ ---
                invZ_t = big.tile([128, HW], f32, name="invZ_t")
                for ch in range(CH):
                    ps_z = ps_acc.tile([128, 512], f32, tag="acc", name="ps_z")
                    for jt in range(TT):
                        mm(
                            ps_z, ones_sb, eT_t[:, jt, ch * 512:(ch + 1) * 512],
                            start=(jt == 0), stop=(jt == TT - 1),
                        )
                    nc.vector.reciprocal(out=invZ_t[:, ch * 512:(ch + 1) * 512], in_=ps_z)

                # --- oT = (v^T @ eT) * invZ + bv  [c, query i] ---
                oT_t = big.tile([128, CT, HW], fdt, name="oT_t")
                for c in range(CT):
                    for ch in range(CH):
                        ps_o = ps_acc.tile([128, 512], f32, tag="acc", name="ps_o")
                        for jt in range(TT):
                            mm(
                                ps_o, v_t[:, jt, c * 128:(c + 1) * 128],
                                eT_t[:, jt, ch * 512:(ch + 1) * 512],
                                start=(jt == 0), stop=(jt == TT - 1),
                            )
                        sl = slice(ch * 512, (ch + 1) * 512)
                        nc.vector.tensor_mul(
                            out=oT_t[:, c, sl], in0=ps_o, in1=invZ_t[:, sl]
                        )

                # --- fT = woT^T @ oT ; y = x + bo + fT ---
                for cp in range(CT):
                    y_t = yout.tile([128, HW], f32, name="y_t")
                    for ch in range(CH):
                        ps_f = ps_acc.tile([128, 512], f32, tag="acc", name="ps_f")
                        for c in range(CT):
                            mm(
                                ps_f, woT_t[:, c, cp * 128:(cp + 1) * 128],
                                oT_t[:, c, ch * 512:(ch + 1) * 512],
                                start=(c == 0), stop=(c == CT - 1),
                            )
                        sl = slice(ch * 512, (ch + 1) * 512)
                        nc.vector.tensor_add(
                            out=y_t[:, sl], in0=ps_f, in1=x_t[:, cp, sl]
                        )
                    nc.sync.dma_start(
                        out=y_d[b, cp * 128:(cp + 1) * 128, :], in_=y_t
                    )
    return nc


def _const_inputs():
    bd = np.zeros((128, 128), np.float32)
    for g in range(128 // G):
        bd[g * G:(g + 1) * G, g * G:(g + 1) * G] = 1.0 / G
    return {"bd16": bd}


def prep_inputs(inputs):
    x = np.ascontiguousarray(np.asarray(inputs["x"], dtype=np.float32)).reshape(B, C, HW)
    wq = np.asarray(inputs["wq"], dtype=np.float32)
    wk = np.asarray(inputs["wk"], dtype=np.float32)
    wv = np.asarray(inputs["wv"], dtype=np.float32)
    wo = np.asarray(inputs["wo"], dtype=np.float32)
    bq = np.asarray(inputs["bq"], dtype=np.float32).reshape(C)
    bv = np.asarray(inputs["bv"], dtype=np.float32).reshape(C)
    bo = np.asarray(inputs["bo"], dtype=np.float32).reshape(C)
    nw = np.asarray(inputs["norm_w"], dtype=np.float32).reshape(C)
    nb = np.asarray(inputs["norm_b"], dtype=np.float32).reshape(C)
    base = dict(_const_inputs())
    base["Wqk"] = np.ascontiguousarray(wq.T @ wk)
    base["wvT"] = np.ascontiguousarray(wv.T)
    base["woT"] = np.ascontiguousarray(wo.T)
    gk = wk.T @ bq
    wob = wo @ bv + bo
    base["vecs"] = np.ascontiguousarray(np.stack([nw, nb, gk, wob], axis=1))
    return base, x


def run_hw(inputs, trace=False):
    from concourse import bacc
    from concourse.bass_utils import run_bass_kernel_spmd

    base, x = prep_inputs(inputs)

    nc = bacc.Bacc("TRN2", target_bir_lowering=False)
    build_program(nc)
    nc.finalize()

    in_maps = [
        {**base, "x": np.ascontiguousarray(x[i * BL:(i + 1) * BL])}
        for i in range(NCORES)
    ]
    try:
        res = run_bass_kernel_spmd(nc, in_maps, list(range(NCORES)), trace=trace)
    except Exception:
        # transient NRT device states (e.g. left over from a prior crashed
        # run) clear on retry
        res = run_bass_kernel_spmd(nc, in_maps, list(range(NCORES)), trace=trace)
    y = np.concatenate([res.results[i]["y"] for i in range(NCORES)], axis=0)
    return y.reshape(B, C, H, W_SP).astype(np.float32), res


def kernel(**inputs):
    y, _ = run_hw(inputs, trace=False)
    return y


# revision 21
# speedup vs baseline: 4.8643x; 1.0610x over previous
"""AttentionBlock (GroupNorm + single-head self-attention + residual) on 8 trn2 cores.

Data-parallel over batch: B=16 -> 2 batch elements per core. Per batch element
(C=512 channels, T=H*W=1024 tokens), everything is kept in channel-major
[C, T] layouts so the whole chain needs zero activation transposes:

  h  = groupnorm(x)                 [C, T]   (bn_stats per channel + block-diag
                                              matmul for cross-partition group agg)
  W  = wq^T @ wk                    [C, C]   (once per core; uses native [O,C] layout)
  u  = W^T @ h  (+ gk := wk^T bq)   [C, T]
  sT = h^T(j) @ u                   [T, T]   scores transposed: [key j, query i]
  eT = exp(sT * C^-1/2)             [T, T]   unnormalized softmax numerator
  Z  = ones^T @ eT                  per-query sums, broadcast to 128 partitions
  oT = (v^T @ eT) * (1/Z) + bv      [C, T]   v = h^T @ wv^T
  fT = wo^T' @ oT                   [C, T]
  y  = x + fT + bo
"""

import numpy as np

B, C, HW = 16, 512, 1024
H = W_SP = 32
G = 16  # channels per group (num_groups=32)
NCORES = 8
BL = B // NCORES  # 2 batch elements per core
CT = C // 128  # 4 channel tiles
TT = HW // 128  # 8 token tiles
CH = HW // 512  # 2 free-dim chunks of 512
EPS = 1e-5
SC = float(C) ** -0.5


def build_program(nc, reps=1, fast=True):
    import concourse.bass as bass
    import concourse.tile as tile
    from concourse import mybir

    f32 = mybir.dt.float32
    f32r = mybir.dt.float32r
    AF = mybir.ActivationFunctionType
    OP = mybir.AluOpType

    # float32r streams 1 row/cycle on the PE (vs 4 for fp32) for N>=256.
    # Tiles feeding f32r matmuls must be written as f32r by their producer op.
    fdt = f32r if fast else f32

    def mm(out, lhsT, rhs, start, stop):
        nc.tensor.matmul(out, lhsT, rhs, start=start, stop=stop)

    x_d = nc.dram_tensor("x", [BL, C, HW], f32, kind="ExternalInput")
    W_d = nc.dram_tensor("Wqk", [C, C], f32, kind="ExternalInput")
    wvT_d = nc.dram_tensor("wvT", [C, C], f32, kind="ExternalInput")
    woT_d = nc.dram_tensor("woT", [C, C], f32, kind="ExternalInput")
    # vecs columns: 0=norm_w 1=norm_b 2=gk(=wk^T bq) 3=wob(=wo bv + bo)
    vec_d = nc.dram_tensor("vecs", [C, 4], f32, kind="ExternalInput")
    bd_d = nc.dram_tensor("bd16", [128, 128], f32, kind="ExternalInput")
    y_d = nc.dram_tensor("y", [BL, C, HW], f32, kind="ExternalOutput")

    with tile.TileContext(nc) as tc:
        with (
            tc.tile_pool(name="persist", bufs=1) as persist,
            tc.tile_pool(name="wtmp", bufs=1) as wtmp,
            tc.tile_pool(name="xin", bufs=2) as xin,
            tc.tile_pool(name="big", bufs=1) as big,
            tc.tile_pool(name="yout", bufs=3) as yout,
            tc.tile_pool(name="small", bufs=2) as small,
            tc.tile_pool(name="ps_score", bufs=2, space="PSUM") as ps_score,
            tc.tile_pool(name="ps_acc", bufs=4, space="PSUM") as ps_acc,
        ):
            # ---------------- startup: weights + constants ----------------
            # All weight algebra (W=wq^T wk, wv^T, wo^T, gk, wob) is done on
            # the host; the device only loads + rounds to f32r. DMAs go on the
            # otherwise-idle gpsimd DGE so x (on SP) lands first.
            bd_sb = persist.tile([128, 128], f32)
            nc.gpsimd.dma_start(out=bd_sb, in_=bd_d[:, :])
            vecs = persist.tile([128, CT, 4], f32)
            for ci in range(CT):
                nc.gpsimd.dma_start(
                    out=vecs[:, ci, :], in_=vec_d[ci * 128:(ci + 1) * 128, :]
                )
            Wf = wtmp.tile([128, CT, C], f32)
            vTf = wtmp.tile([128, CT, C], f32)
            oTf = wtmp.tile([128, CT, C], f32)
            for ci in range(CT):
                sl = slice(ci * 128, (ci + 1) * 128)
                nc.gpsimd.dma_start(out=vTf[:, ci, :], in_=wvT_d[sl, :])
                nc.gpsimd.dma_start(out=Wf[:, ci, :], in_=W_d[sl, :])
                nc.gpsimd.dma_start(out=oTf[:, ci, :], in_=woT_d[sl, :])
            eps_sb = persist.tile([128, 1], f32)
            nc.vector.memset(eps_sb, EPS)
            ones_f = persist.tile([128, 128], f32)
            nc.vector.memset(ones_f, 1.0)
            ones_sb = persist.tile([128, 128], fdt)
            nc.vector.tensor_copy(out=ones_sb, in_=ones_f)

            # round to f32r (structural requirement for f32r matmul operands)
            W_t = persist.tile([128, CT, C], fdt)
            wvT_t = persist.tile([128, CT, C], fdt)
            woT_t = persist.tile([128, CT, C], fdt)
            for ci in range(CT):
                nc.vector.tensor_copy(out=wvT_t[:, ci, :], in_=vTf[:, ci, :])
                nc.vector.tensor_copy(out=W_t[:, ci, :], in_=Wf[:, ci, :])
                nc.gpsimd.tensor_copy(out=woT_t[:, ci, :], in_=oTf[:, ci, :])

            # ---------------- per batch element ----------------
            for b in [b for _ in range(reps) for b in range(BL)]:
                x_t = xin.tile([128, CT, HW], f32, name="x_t")
                for ci in range(CT):
                    for s in range(2):
                        nc.sync.dma_start(
                            out=x_t[:, ci, s * 512:(s + 1) * 512],
                            in_=x_d[b, ci * 128:(ci + 1) * 128, s * 512:(s + 1) * 512],
                        )

                # --- group norm ---
                h_t = big.tile([128, CT, HW], fdt, name="h_t")
                for ci in range(CT):
                    stats = small.tile([128, 2, 6], f32, name="stats")
                    for s in range(2):
                        nc.vector.bn_stats(
                            out=stats[:, s, :], in_=x_t[:, ci, s * 512:(s + 1) * 512]
                        )
                    mv = small.tile([128, 2], f32, name="mv")
                    nc.vector.bn_aggr(out=mv, in_=stats)
                    st2 = small.tile([128, 2], f32, name="st2")
                    nc.vector.tensor_copy(out=st2[:, 0:1], in_=mv[:, 0:1])
                    nc.vector.tensor_mul(out=st2[:, 1:2], in0=mv[:, 0:1], in1=mv[:, 0:1])
                    nc.vector.tensor_add(out=st2[:, 1:2], in0=st2[:, 1:2], in1=mv[:, 1:2])
                    ps_st = ps_acc.tile([128, 2], f32, tag="acc", name="ps_st")
                    nc.tensor.matmul(ps_st, bd_sb, st2, start=True, stop=True)
                    mug = small.tile([128, 1], f32, name="mug")
                    nc.vector.tensor_copy(out=mug, in_=ps_st[:, 0:1])
                    tv = small.tile([128, 1], f32, name="tv")
                    nc.vector.tensor_mul(out=tv, in0=mug, in1=mug)
                    nc.vector.tensor_sub(out=tv, in0=ps_st[:, 1:2], in1=tv)
                    nc.scalar.activation(out=tv, in_=tv, func=AF.Sqrt, bias=eps_sb, scale=1.0)
                    nc.vector.reciprocal(out=tv, in_=tv)
                    sc_c = small.tile([128, 1], f32, name="sc_c")
                    nc.vector.tensor_mul(out=sc_c, in0=tv, in1=vecs[:, ci, 0:1])
                    bi_c = small.tile([128, 1], f32, name="bi_c")
                    nc.vector.tensor_mul(out=bi_c, in0=mug, in1=sc_c)
                    nc.vector.tensor_sub(out=bi_c, in0=vecs[:, ci, 1:2], in1=bi_c)
                    nc.gpsimd.tensor_scalar(
                        out=h_t[:, ci, :], in0=x_t[:, ci, :],
                        scalar1=sc_c, scalar2=bi_c, op0=OP.mult, op1=OP.add,
                    )
                    # x_t becomes (x + bo) for the final residual
                    nc.scalar.activation(
                        out=x_t[:, ci, :], in_=x_t[:, ci, :], func=AF.Identity,
                        bias=vecs[:, ci, 3:4], scale=1.0,
                    )

                # --- v = h^T @ wv^T  [token, c_out] ---
                v_t = big.tile([128, TT, 512], fdt, name="v_t")
                for tt in range(TT):
                    ps_v = ps_acc.tile([128, 512], f32, tag="acc", name="ps_v")
                    for ci in range(CT):
                        mm(
                            ps_v, h_t[:, ci, tt * 128:(tt + 1) * 128], wvT_t[:, ci, :],
                            start=(ci == 0), stop=(ci == CT - 1),
                        )
                    if tt % 2 == 0:
                        nc.scalar.copy(out=v_t[:, tt, :], in_=ps_v)
                    else:
                        nc.vector.tensor_copy(out=v_t[:, tt, :], in_=ps_v)

                # --- u = W^T @ h (+gk)  [cj, query i] ---
                u_t = big.tile([128, CT, HW], fdt, name="u_t")
                for cj in range(CT):
                    for ch in range(CH):
                        ps_u = ps_acc.tile([128, 512], f32, tag="acc", name="ps_u")
                        for ci in range(CT):
                            mm(
                                ps_u, W_t[:, ci, cj * 128:(cj + 1) * 128],
                                h_t[:, ci, ch * 512:(ch + 1) * 512],
                                start=(ci == 0), stop=(ci == CT - 1),
                            )
                        if (cj + ch) % 2 == 0:
                            nc.vector.tensor_scalar_add(
                                out=u_t[:, cj, ch * 512:(ch + 1) * 512], in0=ps_u,
                                scalar1=vecs[:, cj, 2:3],
                            )
                        else:
                            nc.scalar.activation(
                                out=u_t[:, cj, ch * 512:(ch + 1) * 512], in_=ps_u,
                                func=AF.Identity, bias=vecs[:, cj, 2:3], scale=1.0,
                            )

                # --- sT = h^T(j) @ u ; eT = exp(sc * sT) ---
                eT_t = big.tile([128, TT, HW], fdt, name="eT_t")
                for jt in range(TT):
                    ps_s = ps_score.tile([128, CH, 512], f32, name="ps_s")
                    for ch in range(CH):
                        for cj in range(CT):
                            mm(
                                ps_s[:, ch, :], h_t[:, cj, jt * 128:(jt + 1) * 128],
                                u_t[:, cj, ch * 512:(ch + 1) * 512],
                                start=(cj == 0), stop=(cj == CT - 1),
                            )
                    for ch in range(CH):
                        nc.scalar.activation(
                            out=eT_t[:, jt, ch * 512:(ch + 1) * 512], in_=ps_s[:, ch, :],
                            func=AF.Exp, scale=SC,
                        )

                # --- Z = ones^T @ eT (broadcast over partitions), invZ ---
                invZ_t = big.tile([128, HW], f32, name="invZ_t")
                for ch in range(CH):
                    ps_z = ps_acc.tile([128, 512], f32, tag="acc", name="ps_z")
                    for jt in range(TT):
                        mm(
                            ps_z, ones_sb, eT_t[:, jt, ch * 512:(ch + 1) * 512],
                            start=(jt == 0), stop=(jt == TT - 1),
                        )
                    nc.vector.reciprocal(out=invZ_t[:, ch * 512:(ch + 1) * 512], in_=ps_z)

                # --- oT = (v^T @ eT) * invZ + bv  [c, query i] ---
                oT_t = big.tile([128, CT, HW], fdt, name="oT_t")
                for c in range(CT):
                    for ch in range(CH):
                        ps_o = ps_acc.tile([128, 512], f32, tag="acc", name="ps_o")
                        for jt in range(TT):
                            mm(
                                ps_o, v_t[:, jt, c * 128:(c + 1) * 128],
                                eT_t[:, jt, ch * 512:(ch + 1) * 512],
                                start=(jt == 0), stop=(jt == TT - 1),
                            )
                        sl = slice(ch * 512, (ch + 1) * 512)
                        nc.vector.tensor_mul(
                            out=oT_t[:, c, sl], in0=ps_o, in1=invZ_t[:, sl]
                        )

                # --- fT = woT^T @ oT ; y = x + bo + fT ---
                for cp in range(CT):
                    y_t = yout.tile([128, HW], f32, name="y_t")
                    for ch in range(CH):
                        ps_f = ps_acc.tile([128, 512], f32, tag="acc", name="ps_f")
                        for c in range(CT):
                            mm(
                                ps_f, woT_t[:, c, cp * 128:(cp + 1) * 128],
                                oT_t[:, c, ch * 512:(ch + 1) * 512],
                                start=(c == 0), stop=(c == CT - 1),
                            )
                        sl = slice(ch * 512, (ch + 1) * 512)
                        nc.vector.tensor_add(
                            out=y_t[:, sl], in0=ps_f, in1=x_t[:, cp, sl]
                        )
                    nc.sync.dma_start(
                        out=y_d[b, cp * 128:(cp + 1) * 128, :], in_=y_t
                    )
    return nc


def _const_inputs():
    bd = np.zeros((128, 128), np.float32)
    for g in range(128 // G):
        bd[g * G:(g + 1) * G, g * G:(g + 1) * G] = 1.0 / G
    return {"bd16": bd}


def prep_inputs(inputs):
    x = np.ascontiguousarray(np.asarray(inputs["x"], dtype=np.float32)).reshape(B, C, HW)
    wq = np.asarray(inputs["wq"], dtype=np.float32)
    wk = np.asarray(inputs["wk"], dtype=np.float32)
    wv = np.asarray(inputs["wv"], dtype=np.float32)
    wo = np.asarray(inputs["wo"], dtype=np.float32)
    bq = np.asarray(inputs["bq"], dtype=np.float32).reshape(C)
    bv = np.asarray(inputs["bv"], dtype=np.float32).reshape(C)
    bo = np.asarray(inputs["bo"], dtype=np.float32).reshape(C)
    nw = np.asarray(inputs["norm_w"], dtype=np.float32).reshape(C)
    nb = np.asarray(inputs["norm_b"], dtype=np.float32).reshape(C)
    base = dict(_const_inputs())
    base["Wqk"] = np.ascontiguousarray(wq.T @ wk)
    base["wvT"] = np.ascontiguousarray(wv.T)
    base["woT"] = np.ascontiguousarray(wo.T)
    gk = wk.T @ bq
    wob = wo @ bv + bo
    base["vecs"] = np.ascontiguousarray(np.stack([nw, nb, gk, wob], axis=1))
    return base, x


def run_hw(inputs, trace=False):
    from concourse import bacc
    from concourse.bass_utils import run_bass_kernel_spmd

    base, x = prep_inputs(inputs)

    nc = bacc.Bacc("TRN2", target_bir_lowering=False)
    build_program(nc)
    nc.finalize()

    in_maps = [
        {**base, "x": np.ascontiguousarray(x[i * BL:(i + 1) * BL])}
        for i in range(NCORES)
    ]
    try:
        res = run_bass_kernel_spmd(nc, in_maps, list(range(NCORES)), trace=trace)
    except Exception:
        # transient NRT device states (e.g. left over from a prior crashed
        # run) clear on retry
        res = run_bass_kernel_spmd(nc, in_maps, list(range(NCORES)), trace=trace)
    y = np.concatenate([res.results[i]["y"] for i in range(NCORES)], axis=0)
    return y.reshape(B, C, H, W_SP).astype(np.float32), res


def kernel(**inputs):
    y, _ = run_hw(inputs, trace=False)
    return y


# revision 26
# speedup vs baseline: 5.1032x; 1.0491x over previous
"""AttentionBlock (GroupNorm + single-head self-attention + residual) on 8 trn2 cores.

Data-parallel over batch: B=16 -> 2 batch elements per core. Per batch element
(C=512 channels, T=H*W=1024 tokens), everything is kept in channel-major
[C, T] layouts so the whole chain needs zero activation transposes:

  h  = groupnorm(x)                 [C, T]   (bn_stats per channel + block-diag
                                              matmul for cross-partition group agg)
  W  = wq^T @ wk                    [C, C]   (once per core; uses native [O,C] layout)
  u  = W^T @ h  (+ gk := wk^T bq)   [C, T]
  sT = h^T(j) @ u                   [T, T]   scores transposed: [key j, query i]
  eT = exp(sT * C^-1/2)             [T, T]   unnormalized softmax numerator
  Z  = ones^T @ eT                  per-query sums, broadcast to 128 partitions
  oT = (v^T @ eT) * (1/Z) + bv      [C, T]   v = h^T @ wv^T
  fT = wo^T' @ oT                   [C, T]
  y  = x + fT + bo
"""

import numpy as np

B, C, HW = 16, 512, 1024
H = W_SP = 32
G = 16  # channels per group (num_groups=32)
NCORES = 8
BL = B // NCORES  # 2 batch elements per core
CT = C // 128  # 4 channel tiles
TT = HW // 128  # 8 token tiles
CH = HW // 512  # 2 free-dim chunks of 512
EPS = 1e-5
SC = float(C) ** -0.5


def build_program(nc, reps=1, fast=True):
    import concourse.bass as bass
    import concourse.tile as tile
    from concourse import mybir

    f32 = mybir.dt.float32
    f32r = mybir.dt.float32r
    AF = mybir.ActivationFunctionType
    OP = mybir.AluOpType

    # float32r streams 1 row/cycle on the PE (vs 4 for fp32) for N>=256.
    # Tiles feeding f32r matmuls must be written as f32r by their producer op.
    fdt = f32r if fast else f32

    def mm(out, lhsT, rhs, start, stop):
        nc.tensor.matmul(out, lhsT, rhs, start=start, stop=stop)

    x_d = nc.dram_tensor("x", [BL, C, HW], f32, kind="ExternalInput")
    W_d = nc.dram_tensor("Wqk", [C, C], f32, kind="ExternalInput")
    wvT_d = nc.dram_tensor("wvT", [C, C], f32, kind="ExternalInput")
    woT_d = nc.dram_tensor("woT", [C, C], f32, kind="ExternalInput")
    # vecs columns: 0=norm_w 1=norm_b 2=gk(=wk^T bq) 3=wob(=wo bv + bo)
    vec_d = nc.dram_tensor("vecs", [C, 4], f32, kind="ExternalInput")
    bd_d = nc.dram_tensor("bd16", [128, 128], f32, kind="ExternalInput")
    y_d = nc.dram_tensor("y", [BL, C, HW], f32, kind="ExternalOutput")

    with tile.TileContext(nc) as tc:
        with (
            tc.tile_pool(name="persist", bufs=1) as persist,
            tc.tile_pool(name="wtmp", bufs=1) as wtmp,
            tc.tile_pool(name="xin", bufs=2) as xin,
            tc.tile_pool(name="big", bufs=1) as big,
            tc.tile_pool(name="yout", bufs=3) as yout,
            tc.tile_pool(name="small", bufs=2) as small,
            tc.tile_pool(name="ps_score", bufs=2, space="PSUM") as ps_score,
            tc.tile_pool(name="ps_acc", bufs=4, space="PSUM") as ps_acc,
        ):
            # ---------------- startup: weights + constants ----------------
            # Batch-0 x first, at the head of the SP queue: the groupnorm
            # chain is the critical path to the first attention matmul.
            x0_t = xin.tile([128, CT, HW], f32, name="x_t")
            for ci in range(CT):
                for s in range(2):
                    nc.sync.dma_start(
                        out=x0_t[:, ci, s * 512:(s + 1) * 512],
                        in_=x_d[0, ci * 128:(ci + 1) * 128, s * 512:(s + 1) * 512],
                    )
            # Weight algebra (W=wq^T wk, wv^T, wo^T, gk, wob) is done on the
            # host; the device only loads + rounds to f32r. wvT (needed first,
            # for v) rides the short gpsimd queue; W/woT (needed at ~25/55us)
            # go on SP behind x. gpsimd stays free for the GN applies.
            bd_sb = persist.tile([128, 128], f32)
            nc.gpsimd.dma_start(out=bd_sb, in_=bd_d[:, :])
            vecs = persist.tile([128, CT, 4], f32)
            for ci in range(CT):
                nc.gpsimd.dma_start(
                    out=vecs[:, ci, :], in_=vec_d[ci * 128:(ci + 1) * 128, :]
                )
            Wf = wtmp.tile([128, CT, C], f32)
            vTf = wtmp.tile([128, CT, C], f32)
            oTf = wtmp.tile([128, CT, C], f32)
            for ci in range(CT):
                sl = slice(ci * 128, (ci + 1) * 128)
                nc.gpsimd.dma_start(out=vTf[:, ci, :], in_=wvT_d[sl, :])
                nc.sync.dma_start(out=Wf[:, ci, :], in_=W_d[sl, :])
                nc.sync.dma_start(out=oTf[:, ci, :], in_=woT_d[sl, :])
            eps_sb = persist.tile([128, 1], f32)
            nc.vector.memset(eps_sb, EPS)
            ones_f = persist.tile([128, 128], f32)
            nc.vector.memset(ones_f, 1.0)
            ones_sb = persist.tile([128, 128], fdt)
            nc.vector.tensor_copy(out=ones_sb, in_=ones_f)

            # round to f32r (structural requirement for f32r matmul operands)
            W_t = persist.tile([128, CT, C], fdt)
            wvT_t = persist.tile([128, CT, C], fdt)
            woT_t = persist.tile([128, CT, C], fdt)
            for ci in range(CT):
                nc.vector.tensor_copy(out=wvT_t[:, ci, :], in_=vTf[:, ci, :])
                nc.vector.tensor_copy(out=W_t[:, ci, :], in_=Wf[:, ci, :])
                nc.gpsimd.tensor_copy(out=woT_t[:, ci, :], in_=oTf[:, ci, :])

            # ---------------- per batch element ----------------
            for bi, b in enumerate([b for _ in range(reps) for b in range(BL)]):
                if bi == 0:
                    x_t = x0_t
                else:
                    x_t = xin.tile([128, CT, HW], f32, name="x_t")
                    for ci in range(CT):
                        for s in range(2):
                            nc.sync.dma_start(
                                out=x_t[:, ci, s * 512:(s + 1) * 512],
                                in_=x_d[b, ci * 128:(ci + 1) * 128, s * 512:(s + 1) * 512],
                            )

                # --- group norm ---
                h_t = big.tile([128, CT, HW], fdt, name="h_t")
                for ci in range(CT):
                    stats = small.tile([128, 2, 6], f32, name="stats")
                    for s in range(2):
                        nc.vector.bn_stats(
                            out=stats[:, s, :], in_=x_t[:, ci, s * 512:(s + 1) * 512]
                        )
                    mv = small.tile([128, 2], f32, name="mv")
                    nc.vector.bn_aggr(out=mv, in_=stats)
                    st2 = small.tile([128, 2], f32, name="st2")
                    nc.vector.tensor_copy(out=st2[:, 0:1], in_=mv[:, 0:1])
                    nc.vector.tensor_mul(out=st2[:, 1:2], in0=mv[:, 0:1], in1=mv[:, 0:1])
                    nc.vector.tensor_add(out=st2[:, 1:2], in0=st2[:, 1:2], in1=mv[:, 1:2])
                    ps_st = ps_acc.tile([128, 2], f32, tag="acc", name="ps_st")
                    nc.tensor.matmul(ps_st, bd_sb, st2, start=True, stop=True)
                    mug = small.tile([128, 1], f32, name="mug")
                    nc.vector.tensor_copy(out=mug, in_=ps_st[:, 0:1])
                    tv = small.tile([128, 1], f32, name="tv")
                    nc.vector.tensor_mul(out=tv, in0=mug, in1=mug)
                    nc.vector.tensor_sub(out=tv, in0=ps_st[:, 1:2], in1=tv)
                    nc.scalar.activation(out=tv, in_=tv, func=AF.Sqrt, bias=eps_sb, scale=1.0)
                    nc.vector.reciprocal(out=tv, in_=tv)
                    sc_c = small.tile([128, 1], f32, name="sc_c")
                    nc.vector.tensor_mul(out=sc_c, in0=tv, in1=vecs[:, ci, 0:1])
                    bi_c = small.tile([128, 1], f32, name="bi_c")
                    nc.vector.tensor_mul(out=bi_c, in0=mug, in1=sc_c)
                    nc.vector.tensor_sub(out=bi_c, in0=vecs[:, ci, 1:2], in1=bi_c)
                    nc.gpsimd.tensor_scalar(
                        out=h_t[:, ci, :], in0=x_t[:, ci, :],
                        scalar1=sc_c, scalar2=bi_c, op0=OP.mult, op1=OP.add,
                    )
                    # x_t becomes (x + bo) for the final residual
                    nc.scalar.activation(
                        out=x_t[:, ci, :], in_=x_t[:, ci, :], func=AF.Identity,
                        bias=vecs[:, ci, 3:4], scale=1.0,
                    )

                # --- v = h^T @ wv^T  [token, c_out] ---
                v_t = big.tile([128, TT, 512], fdt, name="v_t")
                for tt in range(TT):
                    ps_v = ps_acc.tile([128, 512], f32, tag="acc", name="ps_v")
                    for ci in range(CT):
                        mm(
                            ps_v, h_t[:, ci, tt * 128:(tt + 1) * 128], wvT_t[:, ci, :],
                            start=(ci == 0), stop=(ci == CT - 1),
                        )
                    if tt % 2 == 0:
                        nc.scalar.copy(out=v_t[:, tt, :], in_=ps_v)
                    else:
                        nc.vector.tensor_copy(out=v_t[:, tt, :], in_=ps_v)

                # --- u = W^T @ h (+gk)  [cj, query i] ---
                u_t = big.tile([128, CT, HW], fdt, name="u_t")
                for cj in range(CT):
                    for ch in range(CH):
                        ps_u = ps_acc.tile([128, 512], f32, tag="acc", name="ps_u")
                        for ci in range(CT):
                            mm(
                                ps_u, W_t[:, ci, cj * 128:(cj + 1) * 128],
                                h_t[:, ci, ch * 512:(ch + 1) * 512],
                                start=(ci == 0), stop=(ci == CT - 1),
                            )
                        if (cj + ch) % 2 == 0:
                            nc.vector.tensor_scalar_add(
                                out=u_t[:, cj, ch * 512:(ch + 1) * 512], in0=ps_u,
                                scalar1=vecs[:, cj, 2:3],
                            )
                        else:
                            nc.scalar.activation(
                                out=u_t[:, cj, ch * 512:(ch + 1) * 512], in_=ps_u,
                                func=AF.Identity, bias=vecs[:, cj, 2:3], scale=1.0,
                            )

                # --- sT = h^T(j) @ u ; eT = exp(sc * sT) ---
                eT_t = big.tile([128, TT, HW], fdt, name="eT_t")
                for jt in range(TT):
                    ps_s = ps_score.tile([128, CH, 512], f32, name="ps_s")
                    for ch in range(CH):
                        for cj in range(CT):
                            mm(
                                ps_s[:, ch, :], h_t[:, cj, jt * 128:(jt + 1) * 128],
                                u_t[:, cj, ch * 512:(ch + 1) * 512],
                                start=(cj == 0), stop=(cj == CT - 1),
                            )
                    for ch in range(CH):
                        nc.scalar.activation(
                            out=eT_t[:, jt, ch * 512:(ch + 1) * 512], in_=ps_s[:, ch, :],
                            func=AF.Exp, scale=SC,
                        )

                # --- Z = ones^T @ eT (broadcast over partitions), invZ ---
                invZ_t = big.tile([128, HW], f32, name="invZ_t")
                for ch in range(CH):
                    ps_z = ps_acc.tile([128, 512], f32, tag="acc", name="ps_z")
                    for jt in range(TT):
                        mm(
                            ps_z, ones_sb, eT_t[:, jt, ch * 512:(ch + 1) * 512],
                            start=(jt == 0), stop=(jt == TT - 1),
                        )
                    nc.vector.reciprocal(out=invZ_t[:, ch * 512:(ch + 1) * 512], in_=ps_z)

                # --- oT = (v^T @ eT) * invZ + bv  [c, query i] ---
                oT_t = big.tile([128, CT, HW], fdt, name="oT_t")
                for c in range(CT):
                    for ch in range(CH):
                        ps_o = ps_acc.tile([128, 512], f32, tag="acc", name="ps_o")
                        for jt in range(TT):
                            mm(
                                ps_o, v_t[:, jt, c * 128:(c + 1) * 128],
                                eT_t[:, jt, ch * 512:(ch + 1) * 512],
                                start=(jt == 0), stop=(jt == TT - 1),
                            )
                        sl = slice(ch * 512, (ch + 1) * 512)
                        nc.vector.tensor_mul(
                            out=oT_t[:, c, sl], in0=ps_o, in1=invZ_t[:, sl]
                        )

                # --- fT = woT^T @ oT ; y = x + bo + fT ---
                for cp in range(CT):
                    y_t = yout.tile([128, HW], f32, name="y_t")
                    for ch in range(CH):
                        ps_f = ps_acc.tile([128, 512], f32, tag="acc", name="ps_f")
                        for c in range(CT):
                            mm(
                                ps_f, woT_t[:, c, cp * 128:(cp + 1) * 128],
                                oT_t[:, c, ch * 512:(ch + 1) * 512],
                                start=(c == 0), stop=(c == CT - 1),
                            )
                        sl = slice(ch * 512, (ch + 1) * 512)
                        nc.vector.tensor_add(
                            out=y_t[:, sl], in0=ps_f, in1=x_t[:, cp, sl]
                        )
                        nc.sync.dma_start(
                            out=y_d[b, cp * 128:(cp + 1) * 128, sl], in_=y_t[:, sl]
                        )
    return nc


def _const_inputs():
    bd = np.zeros((128, 128), np.float32)
    for g in range(128 // G):
        bd[g * G:(g + 1) * G, g * G:(g + 1) * G] = 1.0 / G
    return {"bd16": bd}


def prep_inputs(inputs):
    x = np.ascontiguousarray(np.asarray(inputs["x"], dtype=np.float32)).reshape(B, C, HW)
    wq = np.asarray(inputs["wq"], dtype=np.float32)
    wk = np.asarray(inputs["wk"], dtype=np.float32)
    wv = np.asarray(inputs["wv"], dtype=np.float32)
    wo = np.asarray(inputs["wo"], dtype=np.float32)
    bq = np.asarray(inputs["bq"], dtype=np.float32).reshape(C)
    bv = np.asarray(inputs["bv"], dtype=np.float32).reshape(C)
    bo = np.asarray(inputs["bo"], dtype=np.float32).reshape(C)
    nw = np.asarray(inputs["norm_w"], dtype=np.float32).reshape(C)
    nb = np.asarray(inputs["norm_b"], dtype=np.float32).reshape(C)
    base = dict(_const_inputs())
    base["Wqk"] = np.ascontiguousarray(wq.T @ wk)
    base["wvT"] = np.ascontiguousarray(wv.T)
    base["woT"] = np.ascontiguousarray(wo.T)
    gk = wk.T @ bq
    wob = wo @ bv + bo
    base["vecs"] = np.ascontiguousarray(np.stack([nw, nb, gk, wob], axis=1))
    return base, x


def run_hw(inputs, trace=False):
    from concourse import bacc
    from concourse.bass_utils import run_bass_kernel_spmd

    base, x = prep_inputs(inputs)

    nc = bacc.Bacc("TRN2", target_bir_lowering=False)
    build_program(nc)
    nc.finalize()

    in_maps = [
        {**base, "x": np.ascontiguousarray(x[i * BL:(i + 1) * BL])}
        for i in range(NCORES)
    ]
    try:
        res = run_bass_kernel_spmd(nc, in_maps, list(range(NCORES)), trace=trace)
    except Exception:
        # transient NRT device states (e.g. left over from a prior crashed
        # run) clear on retry
        res = run_bass_kernel_spmd(nc, in_maps, list(range(NCORES)), trace=trace)
    y = np.concatenate([res.results[i]["y"] for i in range(NCORES)], axis=0)
    return y.reshape(B, C, H, W_SP).astype(np.float32), res


def kernel(**inputs):
    y, _ = run_hw(inputs, trace=False)
    return y
